# revision 36
# baseline (speedup 1.0000x reference)
"""Trainium2 Bass kernel for nn_DistanceEdgeSelfCond.

Computes, for inputs pred_coords [8,512,3], mask [8,512], W [64,32], b [64]:
    d[i,j]   = ||x_i - x_j||                        (pairwise distances)
    rbf      = exp(coeff * (d - o_k)^2)             (gaussian smearing, K=32)
    edge     = rbf @ W.T + b                        ([B,512,512,64])
    out      = edge * (mask_i * mask_j)[...,None]

Sharding: data-parallel over B — one batch per NeuronCore (8 cores).

Per-core device pipeline:
  1. Gram matmul with host-augmented [5,512] factors -> d^2 in [i,j] layout
     (d^2[i,j] = r_i + r_j - 2 x_i.x_j).
  2. DVE relu*diag-mask + ACT sqrt -> d, compact [128 i, 512 j] x4 chunks.
  3. Per 4-i-row block: one K=128 select+broadcast matmul replicates the 4
     rows of d onto 128 partitions (32 copies each) -> [(i_sub,k), j].
  4. ACT Square(in + (-o_k) per-partition bias) then ACT Exp(scale=coeff)
     -> rbf tile [128, 512], which is exactly the lhsT the edge matmul
     wants.
  5. Edge matmul per 128-pixel j-slice: lhsT = rbf[:, js*128:+128] (K=128),
     rhs = block-diagonal W.T [128, 256] -> [128 px, (i_sub,d)] in PSUM —
     pixel-on-partition, d contiguous: DMA-friendly, no transposes.
  6. DVE tensor_add (fused bias) evacuates PSUM -> SBUF staging; 1 MiB
     contiguous HBM writes.

Walrus's PE LDWEIGHTS struct carries at most ONE sync wait, so:
  - all constants arrive via a single DMA (one wait covers them all),
  - a dummy 1x1 matmul acquires each PSUM tile so slot-release waits land
    on it rather than the real matmul,
  - a post-pass strips PE-self-semaphore waits (redundant: PE completes
    in program order) and relocates any remaining excess wait onto the
    immediately-preceding wait-free PE instruction (sound: gating an
    earlier same-engine instruction is strictly more conservative).
"""

import sys

import numpy as np

for _p in ("/opt/trn_rl_repo", "/root/.axon_site/_ro/trn_rl_repo"):
    if _p not in sys.path:
        sys.path.append(_p)

B = 8
N = 512
K = 32
D = 64
CUTOFF = 10.0

# column offsets inside the merged constant tensor [128, CW]
C_LG = 0          # rows 0:5, cols 0:512
C_RG = 512        # rows 0:5, cols 512:1024
C_NO = 1024       # [128, 1]
C_WC = 1025       # [128, 256]
C_BT = 1281       # [128, 1024]
C_DM = 2305       # [128, 2048]
C_LB = 4353       # [128, 4096]
CW = 8449

_CACHE = {}
TRACE = False  # set True (e.g. from test.py) to capture an NTFF profile


def _fix_waits(nc, mybir):
    """Enforce <=1 embedded sync wait on compute-engine instructions.

    Walrus's per-instruction ISA structs (PE S3_LW, DVE/ACT S2S2D2_*)
    carry a single sync-wait slot.  Excess waits move onto InstNoOp
    instructions inserted immediately before the instruction in the same
    engine stream — gating an earlier point of the same engine is
    strictly more conservative, and with no instruction in between it
    cannot deadlock.
    """
    limited = {
        mybir.EngineType.PE,
        mybir.EngineType.DVE,
        mybir.EngineType.Activation,
        mybir.EngineType.SP,
        mybir.EngineType.Pool,
    }
    for blk in nc.m.functions[0].blocks:
        insts = blk.instructions
        i = 0
        while i < len(insts):
            inst = insts[i]
            si = inst.sync_info
            if (
                inst.engine in limited
                and si is not None
                and si.on_wait
                and len(si.on_wait) > 1
            ):
                waits = list(si.on_wait)
                excess, keep = waits[:-1], waits[-1:]
                for w in excess:
                    nop = mybir.InstNoOp(
                        name=nc.get_next_instruction_name(),
                        sync_info=mybir.SyncInfo(on_wait=[w], on_update=[]),
                        bass_nofuse=True,
                        engine=inst.engine,
                    )
                    nc.register_instruction(nop)
                    insts.insert(i, nop)
                    i += 1
                si.on_wait = keep
            i += 1


def _build_program():
    import concourse.bass as bass
    import concourse.tile as tile
    from concourse import mybir

    f32 = mybir.dt.float32
    bf16 = mybir.dt.bfloat16
    AF = mybir.ActivationFunctionType

    o = np.linspace(0.0, CUTOFF, K)
    coeff = float(-0.5 / (o[1] - o[0]) ** 2)

    nc = bass.Bass("TRN2", target_bir_lowering=False, debug=False)

    ct_d = nc.dram_tensor("ct", [128, CW], f32, kind="ExternalInput")
    out_d = nc.dram_tensor("out", [N, N, D], f32, kind="ExternalOutput")

    out_flat = out_d.ap().rearrange("i j d -> (i j) d")

    with tile.TileContext(nc) as tc:
        with (
            tc.tile_pool(name="consts", bufs=1) as consts,
            tc.tile_pool(name="dtile", bufs=1) as dpool,
            tc.tile_pool(name="work", bufs=2) as work,
            tc.tile_pool(name="stage", bufs=2) as stpool,
            tc.tile_pool(name="psA", bufs=2, space=bass.MemorySpace.PSUM) as psA,
            tc.tile_pool(name="psB", bufs=3, space=bass.MemorySpace.PSUM) as psB,
        ):
            ct_s = consts.tile([128, CW], f32, tag="ct")
            nc.sync.dma_start(ct_s[:], ct_d.ap())
            lg_s = ct_s[0:5, C_LG : C_LG + N]
            rg_s = ct_s[0:5, C_RG : C_RG + N]
            no_s = ct_s[:, C_NO : C_NO + 1]
            wc_s = ct_s[:, C_WC : C_WC + 256]
            bt_s = ct_s[:, C_BT : C_BT + 1024]
            dm_s = ct_s[:, C_DM : C_DM + 2048]
            lb_s = ct_s[:, C_LB : C_LB + 4096]
            # one-time bf16 casts of the matmul constants
            lb_bf = consts.tile([128, 4096], bf16, tag="lbbf")
            nc.vector.tensor_copy(lb_bf[:], lb_s)
            wc_bf = consts.tile([128, 256], bf16, tag="wcbf")
            nc.vector.tensor_copy(wc_bf[:], wc_s)


            # Phase 1: d = sqrt(relu(d^2) * diag0) for all i, [128, 512] x4,
            # stored as a bf16 hi/lo pair (hi + lo reconstructs d to ~2^-18
            # rel inside the fp32 PSUM accumulation of the broadcast matmul).
            dhi_sb = dpool.tile([128, 4 * N], bf16, tag="dhi")
            dlo_sb = dpool.tile([128, 4 * N], bf16, tag="dlo")
            for q in range(4):
                g_ps = psB.tile([128, N], f32, tag="eps")
                nc.tensor.matmul(g_ps[:], lg_s[:, q * 128 : (q + 1) * 128], rg_s)
                draw = work.tile([128, N], f32, tag="draw")
                nc.vector.scalar_tensor_tensor(
                    draw[:],
                    g_ps[:],
                    0.0,
                    dm_s[:, q * N : (q + 1) * N],
                    mybir.AluOpType.max,
                    mybir.AluOpType.mult,
                )
                dfull = work.tile([128, N], f32, tag="dfull")
                nc.scalar.activation(dfull[:], draw[:], AF.Sqrt)
                hi = dhi_sb[:, q * N : (q + 1) * N]
                nc.vector.tensor_copy(hi, dfull[:])
                nc.vector.tensor_sub(dlo_sb[:, q * N : (q + 1) * N], dfull[:], hi)

            # Phase 2: per PAIR of 4-i-row blocks (8 i-rows = one 1 MiB
            # output chunk).  The broadcast matmuls for pair bb+LOOKAHEAD
            # are emitted ahead so PE never stalls on the ACT Square/Exp
            # chain of the current pair.
            diff_tiles = {}

            def emit_bcast(bb):
                # [128, 1024] = 2 PSUM banks; each half holds one 4-i block
                diff = psA.tile([128, 2 * N], f32, tag="diff")
                for bi2 in range(2):
                    q, r = divmod(2 * bb + bi2, 32)
                    half = diff[:, bi2 * N : (bi2 + 1) * N]
                    sel = lb_bf[:, r * 128 : (r + 1) * 128]
                    nc.tensor.matmul(
                        half, sel, dhi_sb[:, q * N : (q + 1) * N],
                        start=True, stop=False,
                    )
                    nc.tensor.matmul(
                        half, sel, dlo_sb[:, q * N : (q + 1) * N],
                        start=False, stop=True,
                    )
                diff_tiles[bb] = diff

            LOOKAHEAD = 1
            for bb in range(LOOKAHEAD):
                emit_bcast(bb)

            for bb in range(N // 8):  # 64 output chunks of 8 i-rows (1 MiB)
                stage = stpool.tile([128, 2048], f32, tag="stage")
                if bb + LOOKAHEAD < N // 8:
                    emit_bcast(bb + LOOKAHEAD)
                diff = diff_tiles.pop(bb)
                sq = work.tile([128, 2 * N], f32, tag="sq")
                nc.scalar.activation(sq[:], diff[:], AF.Square, bias=no_s)
                # edge-matmul lhsT reads rbf quad-strided (col j = 4*pr+e)
                # so each out partition owns 4 consecutive pixels
                # (1 KiB DMA runs).
                rbf = work.tile([128, 2 * N], bf16, tag="rbf")
                nc.scalar.activation(rbf[:], sq[:], AF.Exp, scale=coeff)
                rbf_q = rbf[:].rearrange("p (b pr e) -> p b e pr", b=2, e=4)

                # stage col = i*1024 + g*256 + h*128 + el*64 + d
                # (e = 2*h + el; pixel j = 4*p + e)
                stv = stage[:].rearrange(
                    "p (i g h s d) -> p i h s g d", i=2, g=4, h=2, s=2
                )
                for bi2 in range(2):
                    for h in range(2):
                        # one PSUM bank holds (el, g, d) = 2*4*64
                        eps = psB.tile([128, 512], f32, tag="eps")
                        for el in range(2):
                            e = 2 * h + el
                            nc.tensor.matmul(
                                eps[:, el * 256 : (el + 1) * 256],
                                rbf_q[:, bi2, e, :],
                                wc_bf,
                            )
                        # evacuate + bias (bt is b tiled every 64 -> walk-safe)
                        nc.vector.tensor_add(
                            stv[:, bi2, h], eps[:], bt_s[:, 0:512]
                        )
                # pixel = (8*bb + i_rel)*512 + 4*p + e ; (e,d) contiguous 1 KiB
                dst = (
                    out_flat[bb * 4096 : (bb + 1) * 4096, :]
                    .rearrange("(i pr e) d -> pr i (e d)", i=8, pr=128, e=4)
                )
                eng = nc.sync if bb % 2 == 0 else nc.scalar
                eng.dma_start(
                    dst, stage[:].rearrange("p (i ed) -> p i ed", i=8)
                )

    _fix_waits(nc, mybir)
    return nc


def _host_inputs(pred_coords, W, b):
    o = np.linspace(0.0, CUTOFF, K)

    x64 = pred_coords.astype(np.float64)  # [B, N, 3]
    r = (x64 * x64).sum(-1)  # [B, N]
    ones = np.ones((B, N), np.float64)
    lg = np.stack(
        [x64[:, :, 0], x64[:, :, 1], x64[:, :, 2], r, ones], axis=1
    ).astype(np.float32)  # [B, 5, N]
    rg = np.stack(
        [-2 * x64[:, :, 0], -2 * x64[:, :, 1], -2 * x64[:, :, 2], ones, r],
        axis=1,
    ).astype(np.float32)  # [B, 5, N]

    ct = np.zeros((128, CW), np.float32)

    ct[:, C_NO] = -np.tile(o, 4)

    for g in range(4):
        ct[32 * g : 32 * (g + 1), C_WC + 64 * g : C_WC + 64 * (g + 1)] = W.T

    ct[:, C_BT : C_BT + 1024] = np.tile(b.astype(np.float32), 16)[None, :]

    dm = np.ones((128, 4, N), np.float32)  # diagonal-zero mask per i-chunk
    for q in range(4):
        dm[np.arange(128), q, 128 * q + np.arange(128)] = 0.0
    ct[:, C_DM : C_DM + 2048] = dm.reshape(128, 4 * N)

    p = np.arange(128)
    v = np.arange(32)
    c = np.arange(128)
    lb = (
        p[:, None, None] == 4 * v[None, :, None] + c[None, None, :] // 32
    ).astype(np.float32)  # [128, 32, 128]
    ct[:, C_LB : C_LB + 4096] = lb.reshape(128, 4096)

    cts = []
    for cidx in range(B):
        cc = ct.copy()
        cc[0:5, C_LG : C_LG + N] = lg[cidx]
        cc[0:5, C_RG : C_RG + N] = rg[cidx]
        cts.append(cc)
    return cts


def kernel(pred_coords, mask, W, b):
    from concourse.bass_utils import run_bass_kernel_spmd

    pred_coords = np.asarray(pred_coords)
    mask = np.asarray(mask)
    W = np.asarray(W)
    b = np.asarray(b)

    if "nc" not in _CACHE:
        _CACHE["nc"] = _build_program()
    nc = _CACHE["nc"]

    cts = _host_inputs(pred_coords, W, b)
    in_maps = [{"ct": cts[c]} for c in range(B)]
    import os
    tdir = os.environ.get("KTRACE_DIR") or None
    res = run_bass_kernel_spmd(
        nc, in_maps, list(range(B)), trace=TRACE, tmpdir=tdir
    )
    _CACHE["last_res"] = res
    out = np.stack([res.results[c]["out"] for c in range(B)])  # [B,N,N,64]

    if not np.all(mask == 1.0):
        adj = (mask[:, None, :] * mask[:, :, None]).astype(np.float32)
        out = out * adj[..., None]
    return out


# revision 37
# speedup vs baseline: 1.0885x; 1.0885x over previous
"""Trainium2 Bass kernel for nn_DistanceEdgeSelfCond.

Computes, for inputs pred_coords [8,512,3], mask [8,512], W [64,32], b [64]:
    d[i,j]   = ||x_i - x_j||                        (pairwise distances)
    rbf      = exp(coeff * (d - o_k)^2)             (gaussian smearing, K=32)
    edge     = rbf @ W.T + b                        ([B,512,512,64])
    out      = edge * (mask_i * mask_j)[...,None]

Sharding: data-parallel over B — one batch per NeuronCore (8 cores).

Per-core device pipeline:
  1. Gram matmul with host-augmented [5,512] factors -> d^2 in [i,j] layout
     (d^2[i,j] = r_i + r_j - 2 x_i.x_j).
  2. DVE relu*diag-mask + ACT sqrt -> d, compact [128 i, 512 j] x4 chunks.
  3. Per 4-i-row block: one K=128 select+broadcast matmul replicates the 4
     rows of d onto 128 partitions (32 copies each) -> [(i_sub,k), j].
  4. ACT Square(in + (-o_k) per-partition bias) then ACT Exp(scale=coeff)
     -> rbf tile [128, 512], which is exactly the lhsT the edge matmul
     wants.
  5. Edge matmul per 128-pixel j-slice: lhsT = rbf[:, js*128:+128] (K=128),
     rhs = block-diagonal W.T [128, 256] -> [128 px, (i_sub,d)] in PSUM —
     pixel-on-partition, d contiguous: DMA-friendly, no transposes.
  6. DVE tensor_add (fused bias) evacuates PSUM -> SBUF staging; 1 MiB
     contiguous HBM writes.

Walrus's PE LDWEIGHTS struct carries at most ONE sync wait, so:
  - all constants arrive via a single DMA (one wait covers them all),
  - a dummy 1x1 matmul acquires each PSUM tile so slot-release waits land
    on it rather than the real matmul,
  - a post-pass strips PE-self-semaphore waits (redundant: PE completes
    in program order) and relocates any remaining excess wait onto the
    immediately-preceding wait-free PE instruction (sound: gating an
    earlier same-engine instruction is strictly more conservative).
"""

import sys

import numpy as np

for _p in ("/opt/trn_rl_repo", "/root/.axon_site/_ro/trn_rl_repo"):
    if _p not in sys.path:
        sys.path.append(_p)

B = 8
N = 512
K = 32
D = 64
CUTOFF = 10.0

# column offsets inside the merged constant tensor [128, CW]
C_LG = 0          # rows 0:5, cols 0:512
C_RG = 512        # rows 0:5, cols 512:1024
C_NO = 1024       # [128, 1]
C_WC = 1025       # [128, 256]
C_BT = 1281       # [128, 1024]
C_DM = 2305       # [128, 2048]
C_LB = 4353       # [128, 4096]
CW = 8449

_CACHE = {}
TRACE = False  # set True (e.g. from test.py) to capture an NTFF profile


def _fix_waits(nc, mybir):
    """Enforce <=1 embedded sync wait on compute-engine instructions.

    Walrus's per-instruction ISA structs (PE S3_LW, DVE/ACT S2S2D2_*)
    carry a single sync-wait slot.  Excess waits move onto InstNoOp
    instructions inserted immediately before the instruction in the same
    engine stream — gating an earlier point of the same engine is
    strictly more conservative, and with no instruction in between it
    cannot deadlock.
    """
    limited = {
        mybir.EngineType.PE,
        mybir.EngineType.DVE,
        mybir.EngineType.Activation,
        mybir.EngineType.SP,
        mybir.EngineType.Pool,
    }
    for blk in nc.m.functions[0].blocks:
        insts = blk.instructions
        i = 0
        while i < len(insts):
            inst = insts[i]
            si = inst.sync_info
            if (
                inst.engine in limited
                and si is not None
                and si.on_wait
                and len(si.on_wait) > 1
            ):
                waits = list(si.on_wait)
                excess, keep = waits[:-1], waits[-1:]
                for w in excess:
                    nop = mybir.InstNoOp(
                        name=nc.get_next_instruction_name(),
                        sync_info=mybir.SyncInfo(on_wait=[w], on_update=[]),
                        bass_nofuse=True,
                        engine=inst.engine,
                    )
                    nc.register_instruction(nop)
                    insts.insert(i, nop)
                    i += 1
                si.on_wait = keep
            i += 1


def _build_program():
    import concourse.bass as bass
    import concourse.tile as tile
    from concourse import mybir

    f32 = mybir.dt.float32
    bf16 = mybir.dt.bfloat16
    AF = mybir.ActivationFunctionType

    o = np.linspace(0.0, CUTOFF, K)
    coeff = float(-0.5 / (o[1] - o[0]) ** 2)

    nc = bass.Bass("TRN2", target_bir_lowering=False, debug=False)

    ct_d = nc.dram_tensor("ct", [128, CW], f32, kind="ExternalInput")
    out_d = nc.dram_tensor("out", [N, N, D], f32, kind="ExternalOutput")

    out_flat = out_d.ap().rearrange("i j d -> (i j) d")

    with tile.TileContext(nc) as tc:
        with (
            tc.tile_pool(name="consts", bufs=1) as consts,
            tc.tile_pool(name="dtile", bufs=1) as dpool,
            tc.tile_pool(name="work", bufs=2) as work,
            tc.tile_pool(name="stage", bufs=2) as stpool,
            tc.tile_pool(name="psA", bufs=2, space=bass.MemorySpace.PSUM) as psA,
            tc.tile_pool(name="psB", bufs=3, space=bass.MemorySpace.PSUM) as psB,
        ):
            ct_s = consts.tile([128, CW], f32, tag="ct")
            nc.sync.dma_start(ct_s[:], ct_d.ap())
            lg_s = ct_s[0:5, C_LG : C_LG + N]
            rg_s = ct_s[0:5, C_RG : C_RG + N]
            no_s = ct_s[:, C_NO : C_NO + 1]
            wc_s = ct_s[:, C_WC : C_WC + 256]
            bt_s = ct_s[:, C_BT : C_BT + 1024]
            dm_s = ct_s[:, C_DM : C_DM + 2048]
            lb_s = ct_s[:, C_LB : C_LB + 4096]
            # one-time bf16 casts of the matmul constants
            lb_bf = consts.tile([128, 4096], bf16, tag="lbbf")
            nc.vector.tensor_copy(lb_bf[:], lb_s)
            wc_bf = consts.tile([128, 256], bf16, tag="wcbf")
            nc.vector.tensor_copy(wc_bf[:], wc_s)


            # Phase 1: d = sqrt(relu(d^2) * diag0) for all i, [128, 512] x4,
            # stored as a bf16 hi/lo pair (hi + lo reconstructs d to ~2^-18
            # rel inside the fp32 PSUM accumulation of the broadcast matmul).
            dhi_sb = dpool.tile([128, 4 * N], bf16, tag="dhi")
            dlo_sb = dpool.tile([128, 4 * N], bf16, tag="dlo")
            for q in range(4):
                g_ps = psB.tile([128, N], f32, tag="eps")
                nc.tensor.matmul(g_ps[:], lg_s[:, q * 128 : (q + 1) * 128], rg_s)
                draw = work.tile([128, N], f32, tag="draw")
                nc.vector.scalar_tensor_tensor(
                    draw[:],
                    g_ps[:],
                    0.0,
                    dm_s[:, q * N : (q + 1) * N],
                    mybir.AluOpType.max,
                    mybir.AluOpType.mult,
                )
                dfull = work.tile([128, N], f32, tag="dfull")
                nc.scalar.activation(dfull[:], draw[:], AF.Sqrt)
                hi = dhi_sb[:, q * N : (q + 1) * N]
                nc.vector.tensor_copy(hi, dfull[:])
                nc.vector.tensor_sub(dlo_sb[:, q * N : (q + 1) * N], dfull[:], hi)

            # Phase 2: per PAIR of 4-i-row blocks (8 i-rows = one 1 MiB
            # output chunk).  The broadcast matmuls for pair bb+LOOKAHEAD
            # are emitted ahead so PE never stalls on the ACT Square/Exp
            # chain of the current pair.
            diff_tiles = {}

            def emit_bcast(bb):
                # [128, 1024] = 2 PSUM banks; each half holds one 4-i block
                diff = psA.tile([128, 2 * N], f32, tag="diff")
                for bi2 in range(2):
                    q, r = divmod(2 * bb + bi2, 32)
                    half = diff[:, bi2 * N : (bi2 + 1) * N]
                    sel = lb_bf[:, r * 128 : (r + 1) * 128]
                    nc.tensor.matmul(
                        half, sel, dhi_sb[:, q * N : (q + 1) * N],
                        start=True, stop=False,
                    )
                    nc.tensor.matmul(
                        half, sel, dlo_sb[:, q * N : (q + 1) * N],
                        start=False, stop=True,
                    )
                diff_tiles[bb] = diff

            LOOKAHEAD = 1
            for bb in range(LOOKAHEAD):
                emit_bcast(bb)

            for bb in range(N // 8):  # 64 output chunks of 8 i-rows (1 MiB)
                stage = stpool.tile([128, 2048], f32, tag="stage")
                if bb + LOOKAHEAD < N // 8:
                    emit_bcast(bb + LOOKAHEAD)
                diff = diff_tiles.pop(bb)
                sq = work.tile([128, 2 * N], f32, tag="sq")
                nc.scalar.activation(sq[:], diff[:], AF.Square, bias=no_s)
                # edge-matmul lhsT reads rbf quad-strided (col j = 4*pr+e)
                # so each out partition owns 4 consecutive pixels
                # (1 KiB DMA runs).
                rbf = work.tile([128, 2 * N], bf16, tag="rbf")
                nc.scalar.activation(rbf[:], sq[:], AF.Exp, scale=coeff)
                rbf_q = rbf[:].rearrange("p (b pr e) -> p b e pr", b=2, e=4)

                # stage col = i*1024 + g*256 + h*128 + el*64 + d
                # (e = 2*h + el; pixel j = 4*p + e)
                stv = stage[:].rearrange(
                    "p (i g h s d) -> p i h s g d", i=2, g=4, h=2, s=2
                )
                for bi2 in range(2):
                    for h in range(2):
                        # one PSUM bank holds (el, g, d) = 2*4*64
                        eps = psB.tile([128, 512], f32, tag="eps")
                        for el in range(2):
                            e = 2 * h + el
                            nc.tensor.matmul(
                                eps[:, el * 256 : (el + 1) * 256],
                                rbf_q[:, bi2, e, :],
                                wc_bf,
                            )
                        # evacuate + bias (bt is b tiled every 64 -> walk-safe)
                        nc.vector.tensor_add(
                            stv[:, bi2, h], eps[:], bt_s[:, 0:512]
                        )
                # pixel = (8*bb + i_rel)*512 + 4*p + e ; (e,d) contiguous 1 KiB
                dst = (
                    out_flat[bb * 4096 : (bb + 1) * 4096, :]
                    .rearrange("(i pr e) d -> pr i (e d)", i=8, pr=128, e=4)
                )
                nc.sync.dma_start(
                    dst, stage[:].rearrange("p (i ed) -> p i ed", i=8)
                )

    _fix_waits(nc, mybir)
    return nc


def _host_inputs(pred_coords, W, b):
    o = np.linspace(0.0, CUTOFF, K)

    x64 = pred_coords.astype(np.float64)  # [B, N, 3]
    r = (x64 * x64).sum(-1)  # [B, N]
    ones = np.ones((B, N), np.float64)
    lg = np.stack(
        [x64[:, :, 0], x64[:, :, 1], x64[:, :, 2], r, ones], axis=1
    ).astype(np.float32)  # [B, 5, N]
    rg = np.stack(
        [-2 * x64[:, :, 0], -2 * x64[:, :, 1], -2 * x64[:, :, 2], ones, r],
        axis=1,
    ).astype(np.float32)  # [B, 5, N]

    ct = np.zeros((128, CW), np.float32)

    ct[:, C_NO] = -np.tile(o, 4)

    for g in range(4):
        ct[32 * g : 32 * (g + 1), C_WC + 64 * g : C_WC + 64 * (g + 1)] = W.T

    ct[:, C_BT : C_BT + 1024] = np.tile(b.astype(np.float32), 16)[None, :]

    dm = np.ones((128, 4, N), np.float32)  # diagonal-zero mask per i-chunk
    for q in range(4):
        dm[np.arange(128), q, 128 * q + np.arange(128)] = 0.0
    ct[:, C_DM : C_DM + 2048] = dm.reshape(128, 4 * N)

    p = np.arange(128)
    v = np.arange(32)
    c = np.arange(128)
    lb = (
        p[:, None, None] == 4 * v[None, :, None] + c[None, None, :] // 32
    ).astype(np.float32)  # [128, 32, 128]
    ct[:, C_LB : C_LB + 4096] = lb.reshape(128, 4096)

    cts = []
    for cidx in range(B):
        cc = ct.copy()
        cc[0:5, C_LG : C_LG + N] = lg[cidx]
        cc[0:5, C_RG : C_RG + N] = rg[cidx]
        cts.append(cc)
    return cts


def kernel(pred_coords, mask, W, b):
    from concourse.bass_utils import run_bass_kernel_spmd

    pred_coords = np.asarray(pred_coords)
    mask = np.asarray(mask)
    W = np.asarray(W)
    b = np.asarray(b)

    if "nc" not in _CACHE:
        _CACHE["nc"] = _build_program()
    nc = _CACHE["nc"]

    cts = _host_inputs(pred_coords, W, b)
    in_maps = [{"ct": cts[c]} for c in range(B)]
    import os
    tdir = os.environ.get("KTRACE_DIR") or None
    res = run_bass_kernel_spmd(
        nc, in_maps, list(range(B)), trace=TRACE, tmpdir=tdir
    )
    _CACHE["last_res"] = res
    out = np.stack([res.results[c]["out"] for c in range(B)])  # [B,N,N,64]

    if not np.all(mask == 1.0):
        adj = (mask[:, None, :] * mask[:, :, None]).astype(np.float32)
        out = out * adj[..., None]
    return out


# revision 41
# speedup vs baseline: 1.2423x; 1.1413x over previous
"""Trainium2 Bass kernel for nn_DistanceEdgeSelfCond.

Computes, for inputs pred_coords [8,512,3], mask [8,512], W [64,32], b [64]:
    d[i,j]   = ||x_i - x_j||                        (pairwise distances)
    rbf      = exp(coeff * (d - o_k)^2)             (gaussian smearing, K=32)
    edge     = rbf @ W.T + b                        ([B,512,512,64])
    out      = edge * (mask_i * mask_j)[...,None]

Sharding: data-parallel over B — one batch per NeuronCore (8 cores).

Per-core device pipeline:
  1. Gram matmul with host-augmented [5,512] factors -> d^2 in [i,j] layout
     (d^2[i,j] = r_i + r_j - 2 x_i.x_j).
  2. DVE relu*diag-mask + ACT sqrt -> d, compact [128 i, 512 j] x4 chunks.
  3. Per 4-i-row block: one K=128 select+broadcast matmul replicates the 4
     rows of d onto 128 partitions (32 copies each) -> [(i_sub,k), j].
  4. ACT Square(in + (-o_k) per-partition bias) then ACT Exp(scale=coeff)
     -> rbf tile [128, 512], which is exactly the lhsT the edge matmul
     wants.
  5. Edge matmul per 128-pixel j-slice: lhsT = rbf[:, js*128:+128] (K=128),
     rhs = block-diagonal W.T [128, 256] -> [128 px, (i_sub,d)] in PSUM —
     pixel-on-partition, d contiguous: DMA-friendly, no transposes.
  6. DVE tensor_add (fused bias) evacuates PSUM -> SBUF staging; 1 MiB
     contiguous HBM writes.

Walrus's PE LDWEIGHTS struct carries at most ONE sync wait, so:
  - all constants arrive via a single DMA (one wait covers them all),
  - a dummy 1x1 matmul acquires each PSUM tile so slot-release waits land
    on it rather than the real matmul,
  - a post-pass strips PE-self-semaphore waits (redundant: PE completes
    in program order) and relocates any remaining excess wait onto the
    immediately-preceding wait-free PE instruction (sound: gating an
    earlier same-engine instruction is strictly more conservative).
"""

import sys

import numpy as np

for _p in ("/opt/trn_rl_repo", "/root/.axon_site/_ro/trn_rl_repo"):
    if _p not in sys.path:
        sys.path.append(_p)

B = 8
N = 512
K = 32
D = 64
CUTOFF = 10.0

# column offsets inside the merged constant tensor [128, CW]
C_LG = 0          # rows 0:5, cols 0:512
C_RG = 512        # rows 0:5, cols 512:1024
C_NO = 1024       # [128, 1]
C_WC = 1025       # [128, 256]
C_BT = 1281       # [128, 1024]
C_DM = 2305       # [128, 2048]
C_LB = 4353       # [128, 4096]
CW = 8449

_CACHE = {}
TRACE = False  # set True (e.g. from test.py) to capture an NTFF profile


def _fix_waits(nc, mybir):
    """Enforce <=1 embedded sync wait on compute-engine instructions.

    Walrus's per-instruction ISA structs (PE S3_LW, DVE/ACT S2S2D2_*)
    carry a single sync-wait slot.  Excess waits move onto InstNoOp
    instructions inserted immediately before the instruction in the same
    engine stream — gating an earlier point of the same engine is
    strictly more conservative, and with no instruction in between it
    cannot deadlock.
    """
    limited = {
        mybir.EngineType.PE,
        mybir.EngineType.DVE,
        mybir.EngineType.Activation,
        mybir.EngineType.SP,
        mybir.EngineType.Pool,
    }
    for blk in nc.m.functions[0].blocks:
        insts = blk.instructions
        i = 0
        while i < len(insts):
            inst = insts[i]
            si = inst.sync_info
            if (
                inst.engine in limited
                and si is not None
                and si.on_wait
                and len(si.on_wait) > 1
            ):
                waits = list(si.on_wait)
                excess, keep = waits[:-1], waits[-1:]
                for w in excess:
                    nop = mybir.InstNoOp(
                        name=nc.get_next_instruction_name(),
                        sync_info=mybir.SyncInfo(on_wait=[w], on_update=[]),
                        bass_nofuse=True,
                        engine=inst.engine,
                    )
                    nc.register_instruction(nop)
                    insts.insert(i, nop)
                    i += 1
                si.on_wait = keep
            i += 1


def _build_program():
    import concourse.bass as bass
    import concourse.tile as tile
    from concourse import mybir

    f32 = mybir.dt.float32
    bf16 = mybir.dt.bfloat16
    AF = mybir.ActivationFunctionType

    o = np.linspace(0.0, CUTOFF, K)
    coeff = float(-0.5 / (o[1] - o[0]) ** 2)

    nc = bass.Bass("TRN2", target_bir_lowering=False, debug=False)

    ct_d = nc.dram_tensor("ct", [128, CW], f32, kind="ExternalInput")
    out_d = nc.dram_tensor("out", [N, N, D], f32, kind="ExternalOutput")

    out_flat = out_d.ap().rearrange("i j d -> (i j) d")

    with tile.TileContext(nc) as tc:
        with (
            tc.tile_pool(name="consts", bufs=1) as consts,
            tc.tile_pool(name="dtile", bufs=1) as dpool,
            tc.tile_pool(name="work", bufs=2) as work,
            tc.tile_pool(name="stage", bufs=2) as stpool,
            tc.tile_pool(name="psA", bufs=2, space=bass.MemorySpace.PSUM) as psA,
            tc.tile_pool(name="psB", bufs=4, space=bass.MemorySpace.PSUM) as psB,
        ):
            ct_s = consts.tile([128, CW], f32, tag="ct")
            nc.sync.dma_start(ct_s[:], ct_d.ap())
            lg_s = ct_s[0:5, C_LG : C_LG + N]
            rg_s = ct_s[0:5, C_RG : C_RG + N]
            no_s = ct_s[:, C_NO : C_NO + 1]
            wc_s = ct_s[:, C_WC : C_WC + 256]
            bt_s = ct_s[:, C_BT : C_BT + 1024]
            dm_s = ct_s[:, C_DM : C_DM + 2048]
            lb_s = ct_s[:, C_LB : C_LB + 4096]
            # one-time bf16 casts of the matmul constants
            lb_bf = consts.tile([128, 4096], bf16, tag="lbbf")
            nc.vector.tensor_copy(lb_bf[:], lb_s)
            wc_bf = consts.tile([128, 256], bf16, tag="wcbf")
            nc.vector.tensor_copy(wc_bf[:], wc_s)


            # Phase 1: d = sqrt(relu(d^2) * diag0) for all i, [128, 512] x4,
            # stored as a bf16 hi/lo pair (hi + lo reconstructs d to ~2^-18
            # rel inside the fp32 PSUM accumulation of the broadcast matmul).
            dhi_sb = dpool.tile([128, 4 * N], bf16, tag="dhi")
            dlo_sb = dpool.tile([128, 4 * N], bf16, tag="dlo")
            for q in range(4):
                g_ps = psB.tile([128, N], f32, tag="eps")
                nc.tensor.matmul(g_ps[:], lg_s[:, q * 128 : (q + 1) * 128], rg_s)
                draw = work.tile([128, N], f32, tag="draw")
                nc.vector.scalar_tensor_tensor(
                    draw[:],
                    g_ps[:],
                    0.0,
                    dm_s[:, q * N : (q + 1) * N],
                    mybir.AluOpType.max,
                    mybir.AluOpType.mult,
                )
                dfull = work.tile([128, N], f32, tag="dfull")
                nc.scalar.activation(dfull[:], draw[:], AF.Sqrt)
                hi = dhi_sb[:, q * N : (q + 1) * N]
                nc.vector.tensor_copy(hi, dfull[:])
                nc.vector.tensor_sub(dlo_sb[:, q * N : (q + 1) * N], dfull[:], hi)

            # Phase 2: per PAIR of 4-i-row blocks (8 i-rows = one 1 MiB
            # output chunk).  The broadcast matmuls for pair bb+LOOKAHEAD
            # are emitted ahead so PE never stalls on the ACT Square/Exp
            # chain of the current pair.
            diff_tiles = {}

            def emit_bcast(bb):
                # [128, 1024] = 2 PSUM banks; each half holds one 4-i block
                diff = psA.tile([128, 2 * N], f32, tag="diff")
                for bi2 in range(2):
                    q, r = divmod(2 * bb + bi2, 32)
                    half = diff[:, bi2 * N : (bi2 + 1) * N]
                    sel = lb_bf[:, r * 128 : (r + 1) * 128]
                    nc.tensor.matmul(
                        half, sel, dhi_sb[:, q * N : (q + 1) * N],
                        start=True, stop=False,
                    )
                    nc.tensor.matmul(
                        half, sel, dlo_sb[:, q * N : (q + 1) * N],
                        start=False, stop=True,
                    )
                diff_tiles[bb] = diff

            LOOKAHEAD = 1
            for bb in range(LOOKAHEAD):
                emit_bcast(bb)

            stage = None
            for bb in range(N // 8):  # 64 compute chunks; DMA every 2 (2 MiB)
                if bb % 2 == 0:
                    stage_full = stpool.tile([128, 4096], f32, tag="stage")
                stage = stage_full[:, (bb % 2) * 2048 : (bb % 2 + 1) * 2048]
                if bb + LOOKAHEAD < N // 8:
                    emit_bcast(bb + LOOKAHEAD)
                diff = diff_tiles.pop(bb)
                sq = work.tile([128, 2 * N], f32, tag="sq")
                nc.scalar.activation(sq[:], diff[:], AF.Square, bias=no_s)
                # edge-matmul lhsT reads rbf quad-strided (col j = 4*pr+e)
                # so each out partition owns 4 consecutive pixels
                # (1 KiB DMA runs).
                rbf = work.tile([128, 2 * N], bf16, tag="rbf")
                nc.scalar.activation(rbf[:], sq[:], AF.Exp, scale=coeff)
                rbf_q = rbf[:].rearrange("p (b pr e) -> p b e pr", b=2, e=4)

                # stage col = i*1024 + g*256 + h*128 + el*64 + d
                # (e = 2*h + el; pixel j = 4*p + e)
                stv = stage.rearrange(
                    "p (i g h s d) -> p i h s g d", i=2, g=4, h=2, s=2
                )
                for bi2 in range(2):
                    for h in range(2):
                        # one PSUM bank holds (el, g, d) = 2*4*64
                        eps = psB.tile([128, 512], f32, tag="eps")
                        for el in range(2):
                            e = 2 * h + el
                            nc.tensor.matmul(
                                eps[:, el * 256 : (el + 1) * 256],
                                rbf_q[:, bi2, e, :],
                                wc_bf,
                            )
                        # evacuate + bias (bt is b tiled every 64 -> walk-safe)
                        nc.vector.tensor_add(
                            stv[:, bi2, h], eps[:], bt_s[:, 0:512]
                        )
                # pixel = (8*bb + i_rel)*512 + 4*p + e ; (e,d) contiguous 1 KiB
                if bb % 2 == 1:
                    dst = (
                        out_flat[(bb - 1) * 4096 : (bb + 1) * 4096, :]
                        .rearrange("(i pr e) d -> pr i (e d)", i=16, pr=128, e=4)
                    )
                    nc.sync.dma_start(
                        dst,
                        stage_full[:].rearrange("p (i ed) -> p i ed", i=16),
                    )

    _fix_waits(nc, mybir)
    return nc


def _host_inputs(pred_coords, W, b):
    o = np.linspace(0.0, CUTOFF, K)

    x64 = pred_coords.astype(np.float64)  # [B, N, 3]
    r = (x64 * x64).sum(-1)  # [B, N]
    ones = np.ones((B, N), np.float64)
    lg = np.stack(
        [x64[:, :, 0], x64[:, :, 1], x64[:, :, 2], r, ones], axis=1
    ).astype(np.float32)  # [B, 5, N]
    rg = np.stack(
        [-2 * x64[:, :, 0], -2 * x64[:, :, 1], -2 * x64[:, :, 2], ones, r],
        axis=1,
    ).astype(np.float32)  # [B, 5, N]

    ct = np.zeros((128, CW), np.float32)

    ct[:, C_NO] = -np.tile(o, 4)

    for g in range(4):
        ct[32 * g : 32 * (g + 1), C_WC + 64 * g : C_WC + 64 * (g + 1)] = W.T

    ct[:, C_BT : C_BT + 1024] = np.tile(b.astype(np.float32), 16)[None, :]

    dm = np.ones((128, 4, N), np.float32)  # diagonal-zero mask per i-chunk
    for q in range(4):
        dm[np.arange(128), q, 128 * q + np.arange(128)] = 0.0
    ct[:, C_DM : C_DM + 2048] = dm.reshape(128, 4 * N)

    p = np.arange(128)
    v = np.arange(32)
    c = np.arange(128)
    lb = (
        p[:, None, None] == 4 * v[None, :, None] + c[None, None, :] // 32
    ).astype(np.float32)  # [128, 32, 128]
    ct[:, C_LB : C_LB + 4096] = lb.reshape(128, 4096)

    cts = []
    for cidx in range(B):
        cc = ct.copy()
        cc[0:5, C_LG : C_LG + N] = lg[cidx]
        cc[0:5, C_RG : C_RG + N] = rg[cidx]
        cts.append(cc)
    return cts


def kernel(pred_coords, mask, W, b):
    from concourse.bass_utils import run_bass_kernel_spmd

    pred_coords = np.asarray(pred_coords)
    mask = np.asarray(mask)
    W = np.asarray(W)
    b = np.asarray(b)

    if "nc" not in _CACHE:
        _CACHE["nc"] = _build_program()
    nc = _CACHE["nc"]

    cts = _host_inputs(pred_coords, W, b)
    in_maps = [{"ct": cts[c]} for c in range(B)]
    import os
    tdir = os.environ.get("KTRACE_DIR") or None
    res = run_bass_kernel_spmd(
        nc, in_maps, list(range(B)), trace=TRACE, tmpdir=tdir
    )
    _CACHE["last_res"] = res
    out = np.stack([res.results[c]["out"] for c in range(B)])  # [B,N,N,64]

    if not np.all(mask == 1.0):
        adj = (mask[:, None, :] * mask[:, :, None]).astype(np.float32)
        out = out * adj[..., None]
    return out


# revision 45
# speedup vs baseline: 1.3830x; 1.1132x over previous
"""Trainium2 Bass kernel for nn_DistanceEdgeSelfCond.

Computes, for inputs pred_coords [8,512,3], mask [8,512], W [64,32], b [64]:
    d[i,j]   = ||x_i - x_j||                        (pairwise distances)
    rbf      = exp(coeff * (d - o_k)^2)             (gaussian smearing, K=32)
    edge     = rbf @ W.T + b                        ([B,512,512,64])
    out      = edge * (mask_i * mask_j)[...,None]

Sharding: data-parallel over B — one batch per NeuronCore (8 cores).

Per-core device pipeline:
  1. Gram matmul with host-augmented [5,512] factors -> d^2 in [i,j] layout
     (d^2[i,j] = r_i + r_j - 2 x_i.x_j).
  2. DVE relu*diag-mask + ACT sqrt -> d, compact [128 i, 512 j] x4 chunks.
  3. Per 4-i-row block: one K=128 select+broadcast matmul replicates the 4
     rows of d onto 128 partitions (32 copies each) -> [(i_sub,k), j].
  4. ACT Square(in + (-o_k) per-partition bias) then ACT Exp(scale=coeff)
     -> rbf tile [128, 512], which is exactly the lhsT the edge matmul
     wants.
  5. Edge matmul per 128-pixel j-slice: lhsT = rbf[:, js*128:+128] (K=128),
     rhs = block-diagonal W.T [128, 256] -> [128 px, (i_sub,d)] in PSUM —
     pixel-on-partition, d contiguous: DMA-friendly, no transposes.
  6. DVE tensor_add (fused bias) evacuates PSUM -> SBUF staging; 1 MiB
     contiguous HBM writes.

Walrus's PE LDWEIGHTS struct carries at most ONE sync wait, so:
  - all constants arrive via a single DMA (one wait covers them all),
  - a dummy 1x1 matmul acquires each PSUM tile so slot-release waits land
    on it rather than the real matmul,
  - a post-pass strips PE-self-semaphore waits (redundant: PE completes
    in program order) and relocates any remaining excess wait onto the
    immediately-preceding wait-free PE instruction (sound: gating an
    earlier same-engine instruction is strictly more conservative).
"""

import sys

import numpy as np

for _p in ("/opt/trn_rl_repo", "/root/.axon_site/_ro/trn_rl_repo"):
    if _p not in sys.path:
        sys.path.append(_p)

B = 8
N = 512
K = 32
D = 64
CUTOFF = 10.0

# column offsets inside the merged constant tensor [128, CW]
C_LG = 0          # rows 0:5, cols 0:512
C_RG = 512        # rows 0:5, cols 512:1024
C_NO = 1024       # [128, 1]
C_WC = 1025       # [128, 256]
C_BT = 1281       # [128, 1024]
C_DM = 2305       # [128, 2048]
C_LB = 4353       # [128, 4096]
CW = 8449

_CACHE = {}
TRACE = False  # set True (e.g. from test.py) to capture an NTFF profile


def _fix_waits(nc, mybir):
    """Enforce <=1 embedded sync wait on compute-engine instructions.

    Walrus's per-instruction ISA structs (PE S3_LW, DVE/ACT S2S2D2_*)
    carry a single sync-wait slot.  Excess waits move onto InstNoOp
    instructions inserted immediately before the instruction in the same
    engine stream — gating an earlier point of the same engine is
    strictly more conservative, and with no instruction in between it
    cannot deadlock.
    """
    limited = {
        mybir.EngineType.PE,
        mybir.EngineType.DVE,
        mybir.EngineType.Activation,
        mybir.EngineType.SP,
        mybir.EngineType.Pool,
    }
    for blk in nc.m.functions[0].blocks:
        insts = blk.instructions
        i = 0
        while i < len(insts):
            inst = insts[i]
            si = inst.sync_info
            if (
                inst.engine in limited
                and si is not None
                and si.on_wait
                and len(si.on_wait) > 1
            ):
                waits = list(si.on_wait)
                excess, keep = waits[:-1], waits[-1:]
                for w in excess:
                    nop = mybir.InstNoOp(
                        name=nc.get_next_instruction_name(),
                        sync_info=mybir.SyncInfo(on_wait=[w], on_update=[]),
                        bass_nofuse=True,
                        engine=inst.engine,
                    )
                    nc.register_instruction(nop)
                    insts.insert(i, nop)
                    i += 1
                si.on_wait = keep
            i += 1


def _build_program():
    import concourse.bass as bass
    import concourse.tile as tile
    from concourse import mybir

    f32 = mybir.dt.float32
    bf16 = mybir.dt.bfloat16
    AF = mybir.ActivationFunctionType

    o = np.linspace(0.0, CUTOFF, K)
    coeff = float(-0.5 / (o[1] - o[0]) ** 2)

    nc = bass.Bass("TRN2", target_bir_lowering=False, debug=False)

    ct_d = nc.dram_tensor("ct", [128, CW], f32, kind="ExternalInput")
    out_d = nc.dram_tensor("out", [N, N, D], f32, kind="ExternalOutput")

    out_flat = out_d.ap().rearrange("i j d -> (i j) d")

    with tile.TileContext(nc) as tc:
        with (
            tc.tile_pool(name="consts", bufs=1) as consts,
            tc.tile_pool(name="dtile", bufs=1) as dpool,
            tc.tile_pool(name="work", bufs=3) as work,
            tc.tile_pool(name="stage", bufs=3) as stpool,
            tc.tile_pool(name="psA", bufs=2, space=bass.MemorySpace.PSUM) as psA,
            tc.tile_pool(name="psB", bufs=4, space=bass.MemorySpace.PSUM) as psB,
        ):
            ct_s = consts.tile([128, CW], f32, tag="ct")
            nc.sync.dma_start(ct_s[:], ct_d.ap())
            lg_s = ct_s[0:5, C_LG : C_LG + N]
            rg_s = ct_s[0:5, C_RG : C_RG + N]
            no_s = ct_s[:, C_NO : C_NO + 1]
            wc_s = ct_s[:, C_WC : C_WC + 256]
            bt_s = ct_s[:, C_BT : C_BT + 1024]
            dm_s = ct_s[:, C_DM : C_DM + 2048]
            lb_s = ct_s[:, C_LB : C_LB + 4096]
            # one-time bf16 casts of the matmul constants
            lb_bf = consts.tile([128, 4096], bf16, tag="lbbf")
            nc.vector.tensor_copy(lb_bf[:], lb_s)
            wc_bf = consts.tile([128, 256], bf16, tag="wcbf")
            nc.vector.tensor_copy(wc_bf[:], wc_s)


            # Phase 1: d = sqrt(relu(d^2) * diag0) for all i, [128, 512] x4,
            # stored as a bf16 hi/lo pair (hi + lo reconstructs d to ~2^-18
            # rel inside the fp32 PSUM accumulation of the broadcast matmul).
            dhi = [
                dpool.tile([128, N], bf16, name=f"dhi{q}", tag=f"dhi{q}")
                for q in range(4)
            ]
            dlo = [
                dpool.tile([128, N], bf16, name=f"dlo{q}", tag=f"dlo{q}")
                for q in range(4)
            ]
            for q in range(4):
                g_ps = psB.tile([128, N], f32, tag="eps")
                nc.tensor.matmul(g_ps[:], lg_s[:, q * 128 : (q + 1) * 128], rg_s)
                draw = work.tile([128, N], f32, tag="draw")
                nc.vector.scalar_tensor_tensor(
                    draw[:],
                    g_ps[:],
                    0.0,
                    dm_s[:, q * N : (q + 1) * N],
                    mybir.AluOpType.max,
                    mybir.AluOpType.mult,
                )
                dfull = work.tile([128, N], f32, tag="dfull")
                nc.scalar.activation(dfull[:], draw[:], AF.Sqrt)
                nc.vector.tensor_copy(dhi[q][:], dfull[:])
                nc.vector.tensor_sub(dlo[q][:], dfull[:], dhi[q][:])

            # Phase 2: per PAIR of 4-i-row blocks (8 i-rows = one 1 MiB
            # output chunk).  The broadcast matmuls for pair bb+LOOKAHEAD
            # are emitted ahead so PE never stalls on the ACT Square/Exp
            # chain of the current pair.
            diff_tiles = {}

            def emit_bcast(bb):
                # [128, 1024] = 2 PSUM banks; each half holds one 4-i block
                diff = psA.tile([128, 2 * N], f32, tag="diff")
                for bi2 in range(2):
                    q, r = divmod(2 * bb + bi2, 32)
                    half = diff[:, bi2 * N : (bi2 + 1) * N]
                    sel = lb_bf[:, r * 128 : (r + 1) * 128]
                    nc.tensor.matmul(
                        half, sel, dhi[q][:], start=True, stop=False
                    )
                    nc.tensor.matmul(
                        half, sel, dlo[q][:], start=False, stop=True
                    )
                diff_tiles[bb] = diff

            LOOKAHEAD = 1
            for bb in range(LOOKAHEAD):
                emit_bcast(bb)

            stage = None
            for bb in range(N // 8):  # 64 compute chunks; DMA every 2 (2 MiB)
                if bb % 2 == 0:
                    stage_full = stpool.tile([128, 4096], f32, tag="stage")
                stage = stage_full[:, (bb % 2) * 2048 : (bb % 2 + 1) * 2048]
                if bb + LOOKAHEAD < N // 8:
                    emit_bcast(bb + LOOKAHEAD)
                diff = diff_tiles.pop(bb)
                sq = work.tile([128, 2 * N], f32, tag="sq")
                nc.scalar.activation(sq[:], diff[:], AF.Square, bias=no_s)
                # edge-matmul lhsT reads rbf quad-strided (col j = 4*pr+e)
                # so each out partition owns 4 consecutive pixels
                # (1 KiB DMA runs).
                rbf = work.tile([128, 2 * N], bf16, tag="rbf")
                nc.scalar.activation(rbf[:], sq[:], AF.Exp, scale=coeff)
                rbf_q = rbf[:].rearrange("p (b pr e) -> p b e pr", b=2, e=4)

                # stage col = i*1024 + g*256 + h*128 + el*64 + d
                # (e = 2*h + el; pixel j = 4*p + e)
                stv = stage.rearrange(
                    "p (i g h s d) -> p i h s g d", i=2, g=4, h=2, s=2
                )
                for bi2 in range(2):
                    for h in range(2):
                        # one PSUM bank holds (el, g, d) = 2*4*64
                        eps = psB.tile([128, 512], f32, tag="eps")
                        for el in range(2):
                            e = 2 * h + el
                            nc.tensor.matmul(
                                eps[:, el * 256 : (el + 1) * 256],
                                rbf_q[:, bi2, e, :],
                                wc_bf,
                            )
                        # evacuate + bias (bt is b tiled every 64 -> walk-safe)
                        nc.vector.tensor_add(
                            stv[:, bi2, h], eps[:], bt_s[:, 0:512]
                        )
                # pixel = (8*bb + i_rel)*512 + 4*p + e ; (e,d) contiguous 1 KiB
                if bb % 2 == 1:
                    dst = (
                        out_flat[(bb - 1) * 4096 : (bb + 1) * 4096, :]
                        .rearrange("(i pr e) d -> pr i (e d)", i=16, pr=128, e=4)
                    )
                    nc.sync.dma_start(
                        dst,
                        stage_full[:].rearrange("p (i ed) -> p i ed", i=16),
                    )

    _fix_waits(nc, mybir)
    return nc


def _host_inputs(pred_coords, W, b):
    o = np.linspace(0.0, CUTOFF, K)

    x64 = pred_coords.astype(np.float64)  # [B, N, 3]
    r = (x64 * x64).sum(-1)  # [B, N]
    ones = np.ones((B, N), np.float64)
    lg = np.stack(
        [x64[:, :, 0], x64[:, :, 1], x64[:, :, 2], r, ones], axis=1
    ).astype(np.float32)  # [B, 5, N]
    rg = np.stack(
        [-2 * x64[:, :, 0], -2 * x64[:, :, 1], -2 * x64[:, :, 2], ones, r],
        axis=1,
    ).astype(np.float32)  # [B, 5, N]

    ct = np.zeros((128, CW), np.float32)

    ct[:, C_NO] = -np.tile(o, 4)

    for g in range(4):
        ct[32 * g : 32 * (g + 1), C_WC + 64 * g : C_WC + 64 * (g + 1)] = W.T

    ct[:, C_BT : C_BT + 1024] = np.tile(b.astype(np.float32), 16)[None, :]

    dm = np.ones((128, 4, N), np.float32)  # diagonal-zero mask per i-chunk
    for q in range(4):
        dm[np.arange(128), q, 128 * q + np.arange(128)] = 0.0
    ct[:, C_DM : C_DM + 2048] = dm.reshape(128, 4 * N)

    p = np.arange(128)
    v = np.arange(32)
    c = np.arange(128)
    lb = (
        p[:, None, None] == 4 * v[None, :, None] + c[None, None, :] // 32
    ).astype(np.float32)  # [128, 32, 128]
    ct[:, C_LB : C_LB + 4096] = lb.reshape(128, 4096)

    cts = []
    for cidx in range(B):
        cc = ct.copy()
        cc[0:5, C_LG : C_LG + N] = lg[cidx]
        cc[0:5, C_RG : C_RG + N] = rg[cidx]
        cts.append(cc)
    return cts


def kernel(pred_coords, mask, W, b):
    from concourse.bass_utils import run_bass_kernel_spmd

    pred_coords = np.asarray(pred_coords)
    mask = np.asarray(mask)
    W = np.asarray(W)
    b = np.asarray(b)

    if "nc" not in _CACHE:
        _CACHE["nc"] = _build_program()
    nc = _CACHE["nc"]

    cts = _host_inputs(pred_coords, W, b)
    in_maps = [{"ct": cts[c]} for c in range(B)]
    import os
    tdir = os.environ.get("KTRACE_DIR") or None
    res = run_bass_kernel_spmd(
        nc, in_maps, list(range(B)), trace=TRACE, tmpdir=tdir
    )
    _CACHE["last_res"] = res
    out = np.stack([res.results[c]["out"] for c in range(B)])  # [B,N,N,64]

    if not np.all(mask == 1.0):
        adj = (mask[:, None, :] * mask[:, :, None]).astype(np.float32)
        out = out * adj[..., None]
    return out
